# revision 80
# baseline (speedup 1.0000x reference)
"""Trainium2 Bass kernel for ConvocationV4-style dynamic depthwise conv block.

Data-parallel over batch: 16 samples -> 8 cores x 2 samples. All weights
replicated; per-core program identical (SPMD).

v2 design (vs f32r baseline): bf16 datapath balanced across all 4 engines.
  - x, Wv, Wp in bf16: same PE cost/row as f32r, half the DMA bytes, and
    unlocks DVE 4x (tensor_scalar) / 2x (tensor_tensor) perf modes.
  - dynamic depthwise conv split spatially per (sample, channel-tile):
    leading chunks as PE diag-matmuls (bf16, PSUM-accumulated), trailing
    chunks on DVE as 9 tensor_scalar_mul products (4x mode) + 4 wide
    in-place tree adds (2x mode).
  - PSUM evacuations (value conv, final conv, dw) spread across ACT and
    Pool; diag builds on Pool; q/k pooling reduce on DVE.
  - kerngen identical in structure to the f32r baseline: q-conv commutes
    with block-average pooling; 1/256 and 1/2304 folded into Wq^T/Wk^T;
    biases applied via rank-1 matmuls; sigmoid(k)*mean via a (-1) x (f*m)
    accumulating matmul.
"""

import numpy as np

B, C, H, W, K = 16, 384, 48, 48, 3
NCORES = 8
S = B // NCORES          # samples per core
CT = C // 128            # channel tiles
HW = H * W               # 2304
HP = H + 2               # 50 (padded)
RPC = 8                  # rows per chunk
NCHUNK = H // RPC        # 6 chunks
CHUNK = RPC * W          # 384
BLOBB_W = 780

_CACHE = {}

# ---- tuning knobs ----
DW_C = ((2, 3, 3), (4, 4, 3))   # per (s, ct): leading chunks on PE; rest DVE
# engine cycles for PSUM evacuations ('act' | 'pool' | 'dve'); strict
# alternation keeps two evacs in flight so the PE never waits on a bank
EVAC_VALUE = (("act",), ("act",))
EVAC_FINAL = (("act",), ("act",))
EVAC_PSD = (("act",), ("act",))
DG_ENGINE = ("pool", "pool")    # diag-build engine per sample (SBUF-only)
PROD_ACT = ()                   # (s, ct) units whose dw products go to ACT
Y_QUEUES = ("sync",)            # dma queues for y stores


def _build_nc(dbg=False):
    from contextlib import ExitStack
    import concourse.bacc as bacc
    import concourse.tile as tile
    from concourse import mybir

    f32 = mybir.dt.float32
    f32r = mybir.dt.float32r
    bf16 = mybir.dt.bfloat16
    nc = bacc.Bacc("TRN2", target_bir_lowering=False, debug=False)

    x_d = nc.dram_tensor("x", [S, C, H, W], bf16, kind="ExternalInput")
    # wbig_r: [C, 2C] = [Wv.T | Wp.T] (bf16)
    wr_d = nc.dram_tensor("wbig_r", [C, 2 * C], bf16, kind="ExternalInput")
    # wbig_f: [C, 2C] = [Wq.T/256 | Wk.T/2304] (bf16)
    wf_d = nc.dram_tensor("wbig_f", [C, 2 * C], bf16, kind="ExternalInput")
    # blob_a[128, 134]: cols 0:3 bv, 3:6 bp, 6:134 eye(128)
    ba_d = nc.dram_tensor("blob_a", [128, 134], f32, kind="ExternalInput")
    # blob_b[10, 780]: [0,0:384] bq, [0,384:768] bk, [j,768] gvec, [j,769] bg9,
    # [0,770] bgm, [j,771:780] wg_t
    bb_d = nc.dram_tensor("blob_b", [10, BLOBB_W], f32, kind="ExternalInput")
    y_d = nc.dram_tensor("y", [S, C, H, W], f32, kind="ExternalOutput")
    if dbg:
        dbg_kern = nc.dram_tensor("dbg_kern", [S, C, 9], f32, kind="ExternalOutput")
        dbg_val = nc.dram_tensor("dbg_val", [S, C, HW], f32, kind="ExternalOutput")
        dbg_dw = nc.dram_tensor("dbg_dw", [S, C, HW], f32, kind="ExternalOutput")

    AX = mybir.AxisListType
    OP = mybir.AluOpType
    AF = mybir.ActivationFunctionType

    with tile.TileContext(nc) as tc, ExitStack() as ctx:
        wpool = ctx.enter_context(tc.tile_pool(name="w", bufs=1))
        xpool = ctx.enter_context(tc.tile_pool(name="x", bufs=6))
        vpool = ctx.enter_context(tc.tile_pool(name="vpad", bufs=6))
        scpool = ctx.enter_context(tc.tile_pool(name="scr", bufs=2))
        dpool = ctx.enter_context(tc.tile_pool(name="dwout", bufs=2))
        spool = ctx.enter_context(tc.tile_pool(name="small", bufs=1))
        ypool = ctx.enter_context(tc.tile_pool(name="yout", bufs=4))
        gpool = ctx.enter_context(tc.tile_pool(name="diag", bufs=2))
        ppc = ctx.enter_context(tc.tile_pool(name="ppc", bufs=4, space="PSUM"))
        ppd = ctx.enter_context(tc.tile_pool(name="ppd", bufs=2, space="PSUM"))
        qpool = ctx.enter_context(tc.tile_pool(name="psB", bufs=2, space="PSUM"))

        # round-robin evacuation engine pickers
        def make_cycler(seq):
            state = {"i": 0}

            def pick():
                e = seq[state["i"] % len(seq)]
                state["i"] += 1
                return e
            return pick

        pick_vals = [make_cycler(c) for c in EVAC_VALUE]
        pick_fins = [make_cycler(c) for c in EVAC_FINAL]
        pick_psds = [make_cycler(c) for c in EVAC_PSD]

        def evac_copy(eng, dst, src):
            if eng == "act":
                nc.scalar.copy(dst, src)
            elif eng == "pool":
                nc.gpsimd.tensor_copy(dst, src)
            else:
                nc.vector.tensor_copy(dst, src)

        def evac_bias(eng, dst, src, bias_col):
            if eng == "act":
                nc.scalar.activation(dst, src, AF.Identity, bias=bias_col)
            elif eng == "dve":
                nc.vector.tensor_scalar_add(dst, src, bias_col)
            else:
                nc.gpsimd.tensor_scalar_add(dst, src, bias_col)

        # ---- replicated weights -> SBUF; wv + biases first so the value
        # conv can start as soon as the first x halves land ----
        wr_sb = wpool.tile([128, CT, 2 * C], bf16)
        wrv = wr_d[:].rearrange("(k p) o -> p k o", p=128)
        nc.sync.dma_start(wr_sb[:, :, 0:128], wrv[:, :, 0:128])
        ba_sb = wpool.tile([128, 134], f32)
        nc.scalar.dma_start(ba_sb[:], ba_d[:])
        wf_sb = wpool.tile([128, CT, 2 * C], bf16)
        bb_sb = wpool.tile([10, BLOBB_W], f32)
        eyebf = wpool.tile([128, 128], bf16)
        bqk16 = wpool.tile([1, 2 * C], bf16)

        def load_weights_rest():
            nc.sync.dma_start(wr_sb[:, :, 128:C], wrv[:, :, 128:C])
            nc.sync.dma_start(wf_sb[:], wf_d[:].rearrange("(k p) o -> p k o", p=128))
            nc.sync.dma_start(bb_sb[:], bb_d[:])
            nc.vector.tensor_copy(wgr_sb[:, 0:9], wg_sb)
            nc.vector.tensor_copy(wgr_sb[:, 9:10], gv_sb)
            nc.vector.tensor_copy(eyebf[:], ba_sb[:, 6:134])
            nc.vector.tensor_copy(bqk16[:], bb_sb[0:1, 0:2 * C])

        def wv_l(ki, mo):
            return wr_sb[:, ki, mo * 128:(mo + 1) * 128]

        def wp_l(ki, mo):
            return wr_sb[:, ki, C + mo * 128:C + (mo + 1) * 128]

        bv_sb = ba_sb[:, 0:CT]
        bp_sb = ba_sb[:, CT:2 * CT]
        eye_sb = ba_sb[:, 6:134]
        bq_row = bqk16[0:1, 0:C]
        bk_row = bqk16[0:1, C:2 * C]
        gv_sb = bb_sb[0:9, 768:769]
        bg9_sb = bb_sb[0:9, 769:770]
        bgm_sb = bb_sb[0:1, 770:771]
        wg_sb = bb_sb[0:9, 771:780]

        neg1_sb = wpool.tile([1, 9], bf16)
        nc.vector.memset(neg1_sb[:], -1.0)
        ones9_sb = wpool.tile([1, 9], bf16)
        nc.vector.memset(ones9_sb[:], 1.0)
        pooled = wpool.tile([128, CT, S, 10], bf16)
        wgr_sb = wpool.tile([9, 10], bf16)

        def stage_load(s):
            xs = []
            for ct in range(CT):
                xt = xpool.tile([128, HW], bf16, tag="xs", name=f"xt{s}_{ct}")
                xs.append(xt)
            row_pieces = ((0, 8), (8, 24), (24, 48)) if s == 0 \
                else ((0, 24), (24, 48))
            for (ra, rb) in row_pieces:
                for ct in range(CT):
                    nc.sync.dma_start(
                        xs[ct][:, ra * W:rb * W],
                        x_d[s, ct * 128:(ct + 1) * 128, ra:rb, :].rearrange(
                            "c h w -> c (h w)"))
            return xs

        def stage_pool(s, xs):
            # fold the innermost wi dim with 2x-mode adds before the (1x,
            # input-sized) tensor_reduce: 2460ns -> ~1590ns per (s, ct)
            with nc.allow_low_precision(reason="bf16 pooled sums, ~0.4% rel"):
                for ct in range(CT):
                    fw = scpool.tile([128, 144, 8], bf16, tag="poolf",
                                     bufs=2, name=f"fw{s}_{ct}")
                    xv = xs[ct][:].rearrange("p (r wi) -> p r wi", wi=16)
                    nc.gpsimd.tensor_add(fw[:], xv[:, :, 0:8], xv[:, :, 8:16])
                    nc.vector.tensor_add(fw[:, :, 0:4], fw[:, :, 0:4],
                                         fw[:, :, 4:8])
                    nc.vector.tensor_add(fw[:, :, 0:2], fw[:, :, 0:2],
                                         fw[:, :, 2:4])
                    nc.vector.tensor_reduce(
                        pooled[:, ct, s, 0:9].rearrange(
                            "p (hb wb) -> p hb wb", hb=3),
                        fw[:, :, 0:2].rearrange(
                            "p (hb hi wb) t -> p hb wb hi t", hb=3, hi=16),
                        AX.XY, OP.add)
                    nc.vector.tensor_reduce(
                        pooled[:, ct, s, 9:10], pooled[:, ct, s, 0:9],
                        AX.X, OP.add)

        def stage_value(s, xs):
            vps = []
            for ct in range(CT):
                vp = vpool.tile([128, HP * HP], bf16, tag="vpad",
                                name=f"vp{s}_{ct}")
                v3 = vp[:].rearrange("p (a b) -> p a b", a=HP)
                nc.gpsimd.memset(v3[:, 0, :], 0)
                nc.gpsimd.memset(v3[:, HP - 1, :], 0)
                nc.gpsimd.memset(v3[:, 1:HP - 1, 0], 0)
                nc.gpsimd.memset(v3[:, 1:HP - 1, HP - 1], 0)
                vps.append(vp)
            for mo in range(CT):
                v3 = vps[mo][:].rearrange("p (a b) -> p a b", a=HP)
                for chk in range(NCHUNK):
                    pv = ppc.tile([128, CHUNK], f32, tag="pch", name="pv")
                    for ki in range(CT):
                        nc.tensor.matmul(
                            pv[:], wv_l(ki, mo),
                            xs[ki][:, chk * CHUNK:(chk + 1) * CHUNK],
                            start=(ki == 0), stop=(ki == CT - 1))
                    r0 = chk * RPC
                    evac_bias(pick_vals[s](),
                              v3[:, 1 + r0:1 + r0 + RPC, 1:1 + W],
                              pv[:].rearrange("p (r w) -> p r w", w=W),
                              bv_sb[:, mo:mo + 1])
            return vps

        def stage_kerngen(s, vps):
            qT_ps = qpool.tile([9, C], f32, tag="small")
            for ki in range(CT):
                nc.tensor.matmul(qT_ps[:], pooled[:, ki, s, 0:9],
                                 wf_sb[:, ki, 0:C], start=(ki == 0), stop=False)
            nc.tensor.matmul(qT_ps[:], ones9_sb[:], bq_row,
                             start=False, stop=True)
            kT_ps = qpool.tile([1, C], f32, tag="small")
            for ki in range(CT):
                nc.tensor.matmul(kT_ps[:], pooled[:, ki, s, 9:10],
                                 wf_sb[:, ki, C:2 * C], start=(ki == 0), stop=False)
            nc.tensor.matmul(kT_ps[:], ones9_sb[0:1, 0:1], bk_row,
                             start=False, stop=True)
            qkT_sb = spool.tile([9, C], bf16, tag="qkT")
            nc.scalar.copy(qkT_sb[:], qT_ps[:])
            f_sb = spool.tile([1, C], f32, tag="f")
            nc.scalar.activation(f_sb[:], kT_ps[:], AF.Sigmoid)

            kern_ps = qpool.tile([9, C], f32, tag="small")
            nc.tensor.matmul(kern_ps[:], wgr_sb[:, 0:9], qkT_sb[:],
                             start=True, stop=False)
            mean_ps = qpool.tile([1, C], f32, tag="small")
            nc.tensor.matmul(mean_ps[:], wgr_sb[:, 9:10], qkT_sb[:],
                             start=True, stop=True)
            m_sb = spool.tile([1, C], f32, tag="m")
            nc.scalar.activation(m_sb[:], mean_ps[:], AF.Identity, bias=bgm_sb)
            fm_sb = spool.tile([1, C], bf16, tag="fm")
            nc.vector.tensor_mul(fm_sb[:], f_sb[:], m_sb[:])
            nc.tensor.matmul(kern_ps[:], neg1_sb[:], fm_sb[:],
                             start=False, stop=True)
            kernT_sb = spool.tile([9, C], f32, tag="kernT")
            nc.scalar.activation(kernT_sb[:], kern_ps[:], AF.Identity,
                                 bias=bg9_sb)
            kern_sb = spool.tile([128, CT, 9], f32, tag="kern", bufs=2)
            for ct in range(CT):
                tp2 = qpool.tile([128, 9], f32, tag="small")
                nc.tensor.transpose(tp2[:], kernT_sb[:, ct * 128:(ct + 1) * 128],
                                    eye_sb[0:9, 0:9])
                nc.scalar.copy(kern_sb[:, ct, :], tp2[:])
            if dbg:
                for ct in range(CT):
                    nc.gpsimd.dma_start(dbg_kern[s, ct * 128:(ct + 1) * 128, :],
                                        kern_sb[:, ct, :])
                for ct in range(CT):
                    v3 = vps[ct][:].rearrange("p (a b) -> p a b", a=HP)
                    dv = spool.tile([128, HW], f32, tag="dbgv", bufs=6)
                    nc.vector.tensor_copy(
                        dv[:].rearrange("c (h w) -> c h w", w=W),
                        v3[:, 1:1 + H, 1:1 + W])
                    nc.gpsimd.dma_start(
                        dbg_val[s, ct * 128:(ct + 1) * 128, :], dv[:])
            return kern_sb

        def stage_dw(s, vps, kern_sb):
            dwout = dpool.tile([128, CT, HW], bf16, tag="dwout")
            for ct in range(CT):
                cpe = DW_C[s][ct]
                v3 = vps[ct][:].rearrange("p (a b) -> p a b", a=HP)
                kcols = [kern_sb[:, ct, p:p + 1] for p in range(9)]
                # --- PE share: chunks [0, cpe) via diag matmuls ---
                if cpe > 0:
                    dgs = gpool.tile([128, 9, 128], bf16, tag="dg")
                    for p in range(9):
                        if DG_ENGINE[s] == "act":
                            nc.scalar.activation(dgs[:, p, :], eyebf[:],
                                                 AF.Copy, scale=kcols[p])
                        elif DG_ENGINE[s] == "pool":
                            nc.gpsimd.tensor_scalar_mul(
                                dgs[:, p, :], eyebf[:], kcols[p])
                        else:
                            nc.vector.tensor_scalar_mul(
                                dgs[:, p, :], eyebf[:], kcols[p])
                    for chk in range(cpe):
                        r0 = chk * RPC
                        psd = ppd.tile([128, CHUNK], f32, tag="psd")
                        for p in range(9):
                            i, j = p // 3, p % 3
                            nc.tensor.matmul(
                                psd[:].rearrange("p (h w) -> p h w", w=W),
                                dgs[:, p, :],
                                v3[:, i + r0:i + r0 + RPC, j:j + W],
                                start=(p == 0), stop=(p == 8))
                        evac_copy(pick_psds[s](),
                                  dwout[:, ct, chk * CHUNK:(chk + 1) * CHUNK],
                                  psd[:])
                # --- DVE share: chunks [cpe, 6) as products + tree adds ---
                nr = H - cpe * RPC
                if nr > 0:
                    r0 = cpe * RPC
                    ne = nr * W
                    sc = scpool.tile([128, 9, ne], bf16, tag="sc",
                                     name=f"sc{s}_{ct}")
                    for p in range(9):
                        i, j = p // 3, p % 3
                        src = v3[:, i + r0:i + r0 + nr, j:j + W]
                        if (s, ct) in PROD_ACT:
                            nc.scalar.activation(
                                sc[:, p, :].rearrange("p (h w) -> p h w", w=W),
                                src, AF.Copy, scale=kcols[p])
                        else:
                            nc.vector.tensor_scalar_mul(
                                sc[:, p, :].rearrange("p (h w) -> p h w", w=W),
                                src, kcols[p])
                    # per-chunk add trees so dwout chunks release individually
                    for chk in range(cpe, NCHUNK):
                        cs = slice((chk - cpe) * CHUNK, (chk - cpe + 1) * CHUNK)
                        nc.vector.tensor_add(sc[:, 0:4, cs], sc[:, 0:4, cs],
                                             sc[:, 4:8, cs])
                        nc.vector.tensor_add(sc[:, 0:2, cs], sc[:, 0:2, cs],
                                             sc[:, 2:4, cs])
                        nc.vector.tensor_add(sc[:, 0, cs], sc[:, 0, cs],
                                             sc[:, 1, cs])
                        nc.vector.tensor_add(
                            dwout[:, ct, chk * CHUNK:(chk + 1) * CHUNK],
                            sc[:, 0, cs], sc[:, 8, cs])
            if dbg:
                for ct in range(CT):
                    dd = spool.tile([128, HW], f32, tag="dbgd", bufs=6)
                    nc.vector.tensor_copy(dd[:], dwout[:, ct, :])
                    nc.gpsimd.dma_start(
                        dbg_dw[s, ct * 128:(ct + 1) * 128, :], dd[:])
            return dwout

        def stage_final(s, dwout):
            yts = {}
            # chunk-major so PE-share chunks (ready first) flow immediately
            for chk in range(NCHUNK):
                for mo in range(CT):
                    if chk == 0:
                        yts[mo] = ypool.tile([128, HW], f32, tag="y",
                                             name=f"yt{s}_{mo}")
                    py = ppc.tile([128, CHUNK], f32, tag="pch", name="py")
                    for ki in range(CT):
                        nc.tensor.matmul(
                            py[:], wp_l(ki, mo),
                            dwout[:, ki, chk * CHUNK:(chk + 1) * CHUNK],
                            start=(ki == 0), stop=(ki == CT - 1))
                    evac_bias(pick_fins[s](),
                              yts[mo][:, chk * CHUNK:(chk + 1) * CHUNK],
                              py[:], bp_sb[:, mo:mo + 1])
                if chk % 2 == 1:
                    pair = chk // 2
                    r0 = pair * 2 * RPC
                    for mo in range(CT):
                        yq = Y_QUEUES[(mo + pair) % len(Y_QUEUES)]
                        eng = {"sync": nc.sync, "scalar": nc.scalar,
                               "gpsimd": nc.gpsimd, "vector": nc.vector}[yq]
                        eng.dma_start(
                            y_d[s, mo * 128:(mo + 1) * 128,
                                r0:r0 + 2 * RPC, :].rearrange(
                                    "c h w -> c (h w)"),
                            yts[mo][:, pair * 2 * CHUNK:(pair + 1) * 2 * CHUNK])

        st = {}
        xss = {}
        for s in range(S):
            xss[s] = stage_load(s)
            if s == 0:
                load_weights_rest()
        nc.scalar.dma_start(wr_sb[:, :, C:2 * C], wrv[:, :, C:2 * C])
        for s in range(S):
            xs = xss[s]
            stage_pool(s, xs)
            vps = stage_value(s, xs)
            kern = stage_kerngen(s, vps)
            st[s] = (vps, kern)
            if s > 0:
                dw_prev = stage_dw(s - 1, *st[s - 1])
                stage_final(s - 1, dw_prev)
        dw_last = stage_dw(S - 1, *st[S - 1])
        stage_final(S - 1, dw_last)

    nc.compile()
    return nc


def _get_nc(dbg=False):
    key = ("nc", dbg)
    if key not in _CACHE:
        _CACHE[key] = _build_nc(dbg)
    return _CACHE[key]


def _prep_weights(Wq, bq, Wk, bk, Wv, bv, Wg, bg, Wp, bp):
    import ml_dtypes
    blob_a = np.zeros((128, 134), np.float32)
    blob_a[:, 0:CT] = bv.reshape(CT, 128).T
    blob_a[:, CT:2 * CT] = bp.reshape(CT, 128).T
    blob_a[:, 6:134] = np.eye(128, dtype=np.float32)
    blob_b = np.zeros((10, BLOBB_W), np.float32)
    blob_b[0, 0:C] = bq
    blob_b[0, C:2 * C] = bk
    blob_b[0:9, 768] = Wg.mean(axis=0)
    blob_b[0:9, 769] = bg
    blob_b[0, 770] = bg.mean()
    blob_b[0:9, 771:780] = Wg.T
    return {
        "wbig_r": np.ascontiguousarray(
            np.concatenate([Wv.T, Wp.T], axis=1)).astype(ml_dtypes.bfloat16),
        "wbig_f": np.ascontiguousarray(
            np.concatenate([(Wq / 256.0).T, (Wk / 2304.0).T],
                           axis=1)).astype(ml_dtypes.bfloat16),
        "blob_a": blob_a,
        "blob_b": blob_b,
    }


def kernel(x, Wq, bq, Wk, bk, Wv, bv, Wg, bg, Wp, bp, _trace=False, _dbg=False):
    import ml_dtypes
    from concourse.bass_utils import run_bass_kernel_spmd

    x = np.asarray(x, dtype=np.float32).astype(ml_dtypes.bfloat16)
    x = np.ascontiguousarray(x)
    wmap = _prep_weights(
        *(np.asarray(a, dtype=np.float32)
          for a in (Wq, bq, Wk, bk, Wv, bv, Wg, bg, Wp, bp)))
    in_maps = []
    for c in range(NCORES):
        m = dict(wmap)
        m["x"] = x[c * S:(c + 1) * S]
        in_maps.append(m)
    nc = _get_nc(_dbg)
    res = run_bass_kernel_spmd(nc, in_maps, list(range(NCORES)), trace=_trace)
    y = np.concatenate([res.results[i]["y"] for i in range(NCORES)], axis=0)
    if _trace or _dbg:
        _CACHE["last_results"] = res
    return y
